# revision 40
# baseline (speedup 1.0000x reference)
"""DSVDD retrieval-knn kernel for 8 Trainium2 NeuronCores.

Data-parallel over batch: one image per NeuronCore, no collectives. Per image:

  stage A  3x3 sum-pool of p1/p2 (the /9 of avg-pooling is folded into W),
           bilinear 2x upsample of p2 -> xc channels 0..767 (bf16, DRAM).
           p3 is only pooled: its 1x1-conv runs at native 14x14 resolution
           (psi3 = W3^T pool(p3), TensorE) and the 1792-dim outputs are
           4x-bilinear-upsampled afterwards (DVE/ACT) -> phi3 in DRAM.
           Linearity of resize o 1x1-conv makes this exact and cuts the
           full-resolution contraction from 1794 to 770 channels.
  mm1      phi = W12^T xc + b + phi3 (TensorE bf16; coords as a K=2 chunk;
           bias+phi3 fused into the PSUM eviction via scalar_tensor_tensor).
  mm2      PSUM = 2 phi.C - |C|^2 - |phi|^2 = -dist^2, with the norm terms
           folded in as extra contraction rows (hi/lo bf16 splits; |phi|^2
           row built by a ones-matmul over ACT-squared phi tiles).
  top-3    vector.max (max8) straight on PSUM per 512-column block, then a
           merge max8; vals = sqrt(-top3) (ACT); softmin -> w0*v0 (batched
           over all 25 row-blocks at the end).

Weights are pre-laid-out on the host (bf16 cast, W transpose, 2C, -|C|^2
hi/lo rows, coordinate channels) in make_in_maps; all per-input compute
runs on device.
"""
import os, sys

sys.path.insert(0, os.environ.get("TRN_RL_REPO", "/opt/trn_rl_repo"))

import numpy as np
import ml_dtypes

import concourse.bass as bass
import concourse.tile as tile
from concourse import bacc, mybir

F32 = mybir.dt.float32
BF16 = mybir.dt.bfloat16
AF = mybir.ActivationFunctionType
ALU = mybir.AluOpType
AX = mybir.AxisListType

N_CORES = 8
HW = 3136          # 56*56
NCEN = 3136        # centroids
CO = 1792          # output channels / feature dim
MCH = 14           # CO / 128
HW_CHUNKS = [(i * 512, 512) for i in range(6)] + [(3072, 64)]


# ---------------------------------------------------------------- stage A ---
def _pool3(nc, pool, xv, H, W, out, idx):
    """out[c,h,w] = sum of 3x3 zero-padded neighborhood of xv (no /9).

    xv: [128, H, W] fp32 AP. out: [128, H, W] AP (any dtype).
    """
    a = pool.tile([128, H, W], F32, name=f"pa_{idx}", tag="pa")
    nc.vector.tensor_add(a[:, :, 0:W - 1], xv[:, :, 0:W - 1], xv[:, :, 1:W])
    nc.scalar.copy(a[:, :, W - 1:W], xv[:, :, W - 1:W])
    s = pool.tile([128, H, W], F32, name=f"ps_{idx}", tag="ps")
    nc.vector.tensor_add(s[:, :, 1:W], a[:, :, 1:W], xv[:, :, 0:W - 1])
    nc.scalar.copy(s[:, :, 0:1], a[:, :, 0:1])
    t = pool.tile([128, H, W], F32, name=f"pt_{idx}", tag="pa")
    nc.vector.tensor_add(t[:, 0:H - 1, :], s[:, 0:H - 1, :], s[:, 1:H, :])
    nc.scalar.copy(t[:, H - 1:H, :], s[:, H - 1:H, :])
    nc.vector.tensor_add(out[:, 1:H, :], t[:, 1:H, :], s[:, 0:H - 1, :])
    nc.scalar.copy(out[:, 0:1, :], t[:, 0:1, :])


def _up2_w(nc, pool, q, H, W, out, idx):
    """q [128,H,W] fp32 -> out [128,H,2W], half-pixel bilinear 2x on last axis."""
    x34 = pool.tile([128, H, W], F32, name=f"u2w34_{idx}", tag="s34")
    nc.scalar.mul(x34[:], q[:, :, :], 0.75)
    ov = out.rearrange("p h (a two) -> p h a two", two=2)
    nc.vector.scalar_tensor_tensor(
        out=ov[:, :, 1:W, 0:1], in0=q[:, :, 0:W - 1], scalar=0.25,
        in1=x34[:, :, 1:W], op0=ALU.mult, op1=ALU.add)
    nc.vector.scalar_tensor_tensor(
        out=ov[:, :, 0:W - 1, 1:2], in0=q[:, :, 1:W], scalar=0.25,
        in1=x34[:, :, 0:W - 1], op0=ALU.mult, op1=ALU.add)
    nc.scalar.copy(ov[:, :, 0:1, 0:1], q[:, :, 0:1])
    nc.scalar.copy(ov[:, :, W - 1:W, 1:2], q[:, :, W - 1:W])


def _up2_h(nc, pool, rh, H, W2, out, idx):
    """rh [128,H,W2] fp32 -> out [128,2H,W2] (out may be bf16)."""
    y34 = pool.tile([128, H, W2], F32, name=f"u2h34_{idx}", tag="s34b")
    nc.scalar.mul(y34[:], rh[:, :, :], 0.75)
    ov = out.rearrange("p (a two) w -> p a two w", two=2)
    nc.vector.scalar_tensor_tensor(
        out=ov[:, 1:H, 0:1, :], in0=rh[:, 0:H - 1, :], scalar=0.25,
        in1=y34[:, 1:H, :], op0=ALU.mult, op1=ALU.add)
    nc.vector.scalar_tensor_tensor(
        out=ov[:, 0:H - 1, 1:2, :], in0=rh[:, 1:H, :], scalar=0.25,
        in1=y34[:, 0:H - 1, :], op0=ALU.mult, op1=ALU.add)
    nc.scalar.copy(ov[:, 0:1, 0:1, :], rh[:, 0:1, :])
    nc.scalar.copy(ov[:, H - 1:H, 1:2, :], rh[:, H - 1:H, :])


_PHASES4 = [(7.0, 1.0, 0, 2), (5.0, 3.0, 0, 3), (3.0, 5.0, 1, 0), (1.0, 7.0, 1, 1)]


def _up4_w(nc, pool, q, H, W, out, idx):
    """q [128,H,W] -> out [128,H,4W], half-pixel bilinear 4x on last axis."""
    ov = out.rearrange("p h (a four) -> p h a four", four=4)
    for p, (an, bn, off, lane) in enumerate(_PHASES4):
        tmp = pool.tile([128, H, W - 1], q.dtype, name=f"u4w_{idx}_{p}",
                        tag="tmp4")
        nc.vector.scalar_tensor_tensor(
            out=tmp[:], in0=q[:, :, 0:W - 1], scalar=an / bn,
            in1=q[:, :, 1:W], op0=ALU.mult, op1=ALU.add)
        nc.vector.tensor_scalar_mul(
            ov[:, :, off:off + W - 1, lane:lane + 1], tmp[:], bn / 8.0)
    nc.scalar.copy(ov[:, :, 0:1, 0:1], q[:, :, 0:1])
    nc.scalar.copy(ov[:, :, 0:1, 1:2], q[:, :, 0:1])
    nc.scalar.copy(ov[:, :, W - 1:W, 2:3], q[:, :, W - 1:W])
    nc.scalar.copy(ov[:, :, W - 1:W, 3:4], q[:, :, W - 1:W])


def _up4_h(nc, pool, rh, H, W2, out, idx):
    """rh [128,H,W2] -> out [128,4H,W2] (out may be bf16)."""
    ov = out.rearrange("p (a four) w -> p a four w", four=4)
    for p, (an, bn, off, lane) in enumerate(_PHASES4):
        tmp = pool.tile([128, H - 1, W2], rh.dtype, name=f"u4h_{idx}_{p}",
                        tag="tmp4h")
        nc.vector.scalar_tensor_tensor(
            out=tmp[:], in0=rh[:, 0:H - 1, :], scalar=an / bn,
            in1=rh[:, 1:H, :], op0=ALU.mult, op1=ALU.add)
        nc.vector.tensor_scalar_mul(
            ov[:, off:off + H - 1, lane:lane + 1, :], tmp[:], bn / 8.0)
    nc.scalar.copy(ov[:, 0:1, 0:1, :], rh[:, 0:1, :])
    nc.scalar.copy(ov[:, 0:1, 1:2, :], rh[:, 0:1, :])
    nc.scalar.copy(ov[:, H - 1:H, 2:3, :], rh[:, H - 1:H, :])
    nc.scalar.copy(ov[:, H - 1:H, 3:4, :], rh[:, H - 1:H, :])


def _up4_w_pre(nc, pool, q, H, W, out, idx):
    """Like _up4_w with per-phase output scales folded into ACT-precomputed
    scaled copies of q (no DVE->ACT chaining)."""
    pres = {}
    for c in (1, 3, 5, 7):
        t = pool.tile([128, H, W], q.dtype, name=f"u4wp_{idx}_{c}",
                      tag=f"u4wp{c}", bufs=2)
        nc.scalar.mul(t[:], q[:, :, :], c / 8.0)
        pres[c] = t
    ov = out.rearrange("p h (a four) -> p h a four", four=4)
    for p, (an, bn, off, lane) in enumerate(_PHASES4):
        nc.vector.scalar_tensor_tensor(
            out=ov[:, :, off:off + W - 1, lane:lane + 1],
            in0=q[:, :, 0:W - 1], scalar=an / 8.0,
            in1=pres[int(bn)][:, :, 1:W], op0=ALU.mult, op1=ALU.add)
    nc.scalar.copy(ov[:, :, 0:1, 0:1], q[:, :, 0:1])
    nc.scalar.copy(ov[:, :, 0:1, 1:2], q[:, :, 0:1])
    nc.scalar.copy(ov[:, :, W - 1:W, 2:3], q[:, :, W - 1:W])
    nc.scalar.copy(ov[:, :, W - 1:W, 3:4], q[:, :, W - 1:W])


def _up4_h_pre(nc, pool, rh, H, W2, out, idx):
    """Like _up4_h, but the per-phase output scale is folded into four
    ACT-precomputed scaled copies of rh, so DVE does one stt per phase and
    ACT work has no dependency on DVE results."""
    pres = {}
    for c in (1, 3, 5, 7):
        t = pool.tile([128, H, W2], rh.dtype, name=f"u4hp_{idx}_{c}",
                      tag=f"u4hp{c}", bufs=2)
        nc.scalar.mul(t[:], rh[:, :, :], c / 8.0)
        pres[c] = t
    ov = out.rearrange("p (a four) w -> p a four w", four=4)
    for p, (an, bn, off, lane) in enumerate(_PHASES4):
        nc.vector.scalar_tensor_tensor(
            out=ov[:, off:off + H - 1, lane:lane + 1, :],
            in0=rh[:, 0:H - 1, :], scalar=an / 8.0,
            in1=pres[int(bn)][:, 1:H, :], op0=ALU.mult, op1=ALU.add)
    nc.scalar.copy(ov[:, 0:1, 0:1, :], rh[:, 0:1, :])
    nc.scalar.copy(ov[:, 0:1, 1:2, :], rh[:, 0:1, :])
    nc.scalar.copy(ov[:, H - 1:H, 2:3, :], rh[:, H - 1:H, :])
    nc.scalar.copy(ov[:, H - 1:H, 3:4, :], rh[:, H - 1:H, :])


def _stage_a(nc, tc, p1, p2, p3, xc, wt_sb, phi3, rep=0):
    """Produce xc[k] (k=0..5: pooled p1 + pooled-and-resized p2, bf16) and
    phi3[m] = up4(W3^T . pool3(p3)) in DRAM.

    The p3 branch contracts at native 14x14 resolution (mm1b) and upsamples
    the 1792-dim outputs afterwards -- linearity makes this exact and it cuts
    the big mm1 contraction from 1794 to 770 channels.
    """
    with tc.tile_pool(name=f"sa{rep}", bufs=1) as pool, \
         tc.tile_pool(name=f"ps3{rep}", bufs=2, space="PSUM") as ps3p:
        q3_sb = pool.tile([128, 8, 196], BF16, name=f"q3_{rep}", tag="q3")

        def p3_pool(c):
            x = pool.tile([128, 196], F32, name=f"x3_{c}", tag="xload23",
                          bufs=2)
            nc.gpsimd.dma_start(x[:], p3.ap()[c * 128:(c + 1) * 128, :])
            _pool3(nc, pool,
                   x[:].rearrange("p (h w) -> p h w", w=14), 14, 14,
                   q3_sb[:, c, :].rearrange("p (h w) -> p h w", w=14),
                   f"p3_{c}")

        def p1_chunk(c):
            x = pool.tile([128, 3136], F32, name=f"x1_{c}", tag="xload1")
            nc.gpsimd.dma_start(x[:], p1.ap()[c * 128:(c + 1) * 128, :])
            xcbf = pool.tile([128, 56, 56], BF16, name=f"xc1_{c}", tag="xcbf",
                             bufs=2)
            _pool3(nc, pool, x[:].rearrange("p (h w) -> p h w", w=56), 56, 56,
                   xcbf[:], f"p1_{c}")
            nc.sync.dma_start(
                xc.ap()[c], xcbf[:].rearrange("p h w -> p (h w)"))

        def p2_chunk(c):
            x = pool.tile([128, 784], F32, name=f"x2_{c}", tag="xload23",
                          bufs=2)
            nc.gpsimd.dma_start(x[:], p2.ap()[c * 128:(c + 1) * 128, :])
            q = pool.tile([128, 28, 28], F32, name=f"q2_{c}", tag="q")
            _pool3(nc, pool, x[:].rearrange("p (h w) -> p h w", w=28), 28, 28,
                   q[:], f"p2_{c}")
            rh = pool.tile([128, 28, 56], F32, name=f"rh2_{c}", tag="rh")
            _up2_w(nc, pool, q[:], 28, 28, rh[:], f"p2_{c}")
            xcbf = pool.tile([128, 56, 56], BF16, name=f"xc2_{c}", tag="xcbf",
                             bufs=2)
            _up2_h(nc, pool, rh[:], 28, 56, xcbf[:], f"p2_{c}")
            nc.sync.dma_start(
                xc.ap()[2 + c], xcbf[:].rearrange("p h w -> p (h w)"))

        def psi_step(m):
            # psi3 = W3^T q3 (PE), then 4x upsample on DVE -> phi3 DRAM
            ps3 = ps3p.tile([128, 196], F32, name=f"ps3_{m}", tag="ps3",
                            padded_shape=[128, 512])
            msl = slice(m * 128, (m + 1) * 128)
            for k8 in range(8):
                nc.tensor.matmul(ps3[:], wt_sb[:, 6 + k8, msl],
                                 q3_sb[:, k8, :],
                                 start=(k8 == 0), stop=(k8 == 7))
            psi = pool.tile([128, 196], BF16, name=f"psi_{m}", tag="psi",
                            bufs=2)
            nc.scalar.copy(psi[:], ps3[:])
            rh3 = pool.tile([128, 14, 56], BF16, name=f"rh3_{m}", tag="rh3",
                            bufs=2)
            _up4_w_pre(nc, pool, psi[:].rearrange("p (h w) -> p h w", w=14),
                       14, 14, rh3[:], f"ps_{m}")
            phi3m = pool.tile([128, 56, 56], BF16, name=f"phi3_{m}",
                              tag="xcbf", bufs=2)
            _up4_h_pre(nc, pool, rh3[:], 14, 56, phi3m[:], f"ps_{m}")
            nc.sync.dma_start(
                phi3.ap()[m], phi3m[:].rearrange("p h w -> p (h w)"))

        # Interleave the psi chain between the p1/p2 chunks so phi3 is done
        # by the time xc completes and mm1a can start immediately after.
        for c in range(8):
            p3_pool(c)
        psi_step(0); psi_step(1)
        p1_chunk(0)
        psi_step(2); psi_step(3)
        p1_chunk(1)
        psi_step(4); psi_step(5)
        p2_chunk(0)
        psi_step(6); psi_step(7)
        p2_chunk(1)
        psi_step(8); psi_step(9)
        p2_chunk(2)
        psi_step(10); psi_step(11)
        p2_chunk(3)
        psi_step(12); psi_step(13)


# ------------------------------------------------------------ full program ---
def build_program(debug_xc=False, repeat=1):
    nc = bacc.Bacc("TRN2", target_bir_lowering=False, debug=False,
                   num_devices=N_CORES)

    p1 = nc.dram_tensor("p1", (256, HW), F32, kind="ExternalInput")
    p2 = nc.dram_tensor("p2", (512, 784), F32, kind="ExternalInput")
    p3 = nc.dram_tensor("p3", (1024, 196), F32, kind="ExternalInput")
    wt = nc.dram_tensor("wt", (MCH, 128, CO), BF16, kind="ExternalInput")
    wt2 = nc.dram_tensor("wt2", (2, CO), BF16, kind="ExternalInput")
    bias = nc.dram_tensor("bias", (MCH, 128), F32, kind="ExternalInput")
    c2 = nc.dram_tensor("c2", (MCH, 128, NCEN), BF16, kind="ExternalInput")
    caug = nc.dram_tensor("caug", (4, NCEN), BF16, kind="ExternalInput")
    coords = nc.dram_tensor("coords", (2, HW), BF16, kind="ExternalInput")
    score = nc.dram_tensor("score", (HW,), F32, kind="ExternalOutput")
    # xc chunks 0..5: pooled p1 / resized p2; chunk 6 rows 0:2: coords.
    xcs, phi3s = [], []
    for r in range(repeat):
        kind = ("ExternalOutput" if (debug_xc and r == 0) else "Internal")
        xcs.append(nc.dram_tensor(f"xc{r}" if r else "xc",
                                  (7, 128, HW), BF16, kind=kind))
        phi3s.append(nc.dram_tensor(f"phi3_{r}" if r else "phi3",
                                    (MCH, 128, HW), BF16, kind=kind))
    scores = [nc.dram_tensor(f"score_dummy{r}", (HW,), F32, kind="Internal")
              for r in range(repeat - 1)] + [score]

    with tile.TileContext(nc) as tc:
        with tc.tile_pool(name="persist", bufs=1) as pp:
            wt_sb = pp.tile([128, MCH, CO], BF16, name="wt_sb")
            nc.sync.dma_start(wt_sb[:], wt.ap().rearrange("k p o -> p k o"))
            wt2_sb = pp.tile([2, CO], BF16, name="wt2_sb")
            nc.sync.dma_start(wt2_sb[:], wt2.ap())
            bias_sb = pp.tile([128, MCH], F32, name="bias_sb")
            nc.sync.dma_start(bias_sb[:], bias.ap().rearrange("m p -> p m"))
            caug_sb = pp.tile([4, NCEN], BF16, name="caug_sb")
            nc.sync.dma_start(caug_sb[:], caug.ap())
            ones2 = pp.tile([128, 2], BF16, name="ones2")
            nc.vector.memset(ones2[:], 1.0)
            score_sb = pp.tile([128, 25], F32, name="score_sb")
            # first half of the centroid bank loads during stage A; the rest
            # streams in once the stage-A pool frees its SBUF range.
            c2a_sb = pp.tile([128, 7, NCEN], BF16, name="c2a_sb")
            nc.sync.dma_start(
                c2a_sb[:], c2.ap()[0:7].rearrange("k p j -> p k j"))

            for rep in range(repeat):
                _run_once(nc, tc, rep, p1, p2, p3, coords, c2, xcs[rep],
                          phi3s[rep], scores[rep], wt_sb, wt2_sb, bias_sb,
                          caug_sb, ones2, score_sb, c2a_sb)

    nc.compile()
    return nc


def _run_once(nc, tc, rep, p1, p2, p3, coords, c2, xc, phi3, score, wt_sb,
              wt2_sb, bias_sb, caug_sb, ones2, score_sb, c2a_sb):
    if True:
        if True:
            nc.sync.dma_start(xc.ap()[6, 0:2, :], coords.ap())
            _stage_a(nc, tc, p1, p2, p3, xc, wt_sb, phi3, rep)

            with tc.tile_pool(name=f"mm{rep}", bufs=2) as mp, \
                 tc.tile_pool(name=f"tail{rep}", bufs=2) as tp, \
                 tc.tile_pool(name=f"ps_phi{rep}", bufs=2, space="PSUM") as pph, \
                 tc.tile_pool(name=f"ps_n{rep}", bufs=2, space="PSUM") as pn, \
                 tc.tile_pool(name=f"ps_d{rep}", bufs=3, space="PSUM") as pd:
                c2b_sb = mp.tile([128, MCH - 7, NCEN], BF16, name="c2b_sb",
                                 bufs=1)
                nc.sync.dma_start(
                    c2b_sb[:], c2.ap()[7:MCH].rearrange("k p j -> p k j"))

                def c2_ap(k, jsl):
                    return (c2a_sb[:, k, jsl] if k < 7
                            else c2b_sb[:, k - 7, jsl])

                top8_all = tp.tile([128, 25, 8], F32, name=f"t8a_{rep}",
                                   tag="t8all", bufs=1)
                nc.vector.memset(top8_all[:], 0.0)
                for ci, (c0, w) in enumerate(HW_CHUNKS):
                    sl = slice(c0, c0 + w)
                    xc_t = mp.tile([128, 7, w], BF16, name=f"xct_{ci}",
                                   tag="xct", padded_shape=[128, 7, 512])
                    nc.sync.dma_start(
                        xc_t[:], xc.ap()[:, :, sl].rearrange("k p n -> p k n"))
                    phi3_t = mp.tile([128, MCH, w], BF16, name=f"p3t_{ci}",
                                     tag="phi3t", padded_shape=[128, MCH, 512],
                                     bufs=1)
                    nc.sync.dma_start(
                        phi3_t[:],
                        phi3.ap()[:, :, sl].rearrange("k p n -> p k n"))
    # ---- mm1 (phi = W^T xc + b) with the |phi|^2 ones-matmuls interleaved
                    # one m-step behind, so the ACT evict+square chain stays
                    # ahead of PE and never stalls it.
                    phi_sb = mp.tile([128, MCH, w], BF16, name=f"phi_{ci}",
                                     tag="phi", padded_shape=[128, MCH, 512])
                    ps_n = pn.tile([2, w], F32, name=f"psn_{ci}", tag="psn",
                                   padded_shape=[2, 512])
                    phisqs = [None] * MCH

                    def emit_ones_mm(m):
                        nc.tensor.matmul(ps_n[:], ones2[:], phisqs[m][:],
                                         start=(m == 0), stop=(m == MCH - 1),
                                         skip_group_check=True)

                    for m in range(MCH):
                        ph = pph.tile([128, w], F32, name=f"ph_{ci}_{m}",
                                      tag="ph", padded_shape=[128, 512])
                        msl = slice(m * 128, (m + 1) * 128)
                        for k in range(6):
                            nc.tensor.matmul(ph[:], wt_sb[:, k, msl],
                                             xc_t[:, k, :],
                                             start=(k == 0), stop=False)
                        nc.tensor.matmul(ph[:], wt2_sb[:, msl],
                                         xc_t[0:2, 6, :],
                                         start=False, stop=True)
                        # evict: phi = (psum + b) + phi3, cast to bf16 (DVE)
                        nc.vector.scalar_tensor_tensor(
                            out=phi_sb[:, m, :], in0=ph[:],
                            scalar=bias_sb[:, m:m + 1], in1=phi3_t[:, m, :],
                            op0=ALU.add, op1=ALU.add)
                        phisq = mp.tile([128, w], BF16, name=f"phsq_{ci}_{m}",
                                        tag="phisq", padded_shape=[128, 512],
                                        bufs=2)
                        nc.scalar.activation(phisq[:], phi_sb[:, m, :],
                                             AF.Square)
                        phisqs[m] = phisq
                        if m >= 1:
                            emit_ones_mm(m - 1)
                    emit_ones_mm(MCH - 1)
                    # aug rows: [hi|phi|^2, lo|phi|^2, 1, 1] built from ps_n
                    # (rows 0,1 of ps_n are identical). Compute-ops may not
                    # address partition base 1, so the lo row is placed via a
                    # tiny SBUF->SBUF DMA.
                    aug = mp.tile([4, w], BF16, name=f"aug_{ci}", tag="aug",
                                  padded_shape=[4, 512], bufs=1)
                    nc.vector.memset(aug[:], 1.0)
                    nc.scalar.copy(aug[0:2, :], ps_n[0:2, :])
                    lo2 = mp.tile([2, w], BF16, name=f"lo2_{ci}", tag="phisq",
                                  padded_shape=[2, 512], bufs=2)
                    nc.vector.tensor_sub(lo2[:], ps_n[0:2, :], aug[0:2, :])
                    nc.sync.dma_start(aug[1:2, :], lo2[1:2, :])
                    # ---- mm2 + top-k + tail per 128-row block
                    nblk = (w + 127) // 128
                    for blk in range(nblk):
                        mblk = min(128, w - blk * 128)
                        bsl = slice(blk * 128, blk * 128 + mblk)
                        m8 = tp.tile([128, 56], F32, name=f"m8_{ci}_{blk}",
                                     tag="m8", bufs=2)
                        for j, (j0, wj) in enumerate(HW_CHUNKS):
                            jsl = slice(j0, j0 + wj)
                            pdt = pd.tile([128, wj], F32,
                                          name=f"pd_{ci}_{blk}_{j}", tag="pd",
                                          padded_shape=[128, 512])
                            for k in range(MCH):
                                nc.tensor.matmul(pdt[0:mblk, :],
                                                 phi_sb[:, k, bsl],
                                                 c2_ap(k, jsl),
                                                 start=(k == 0), stop=False)
                            nc.tensor.matmul(pdt[0:mblk, :], aug[:, bsl],
                                             caug_sb[:, jsl],
                                             start=False, stop=True)
                            nc.vector.max(out=m8[0:mblk, j * 8:(j + 1) * 8],
                                          in_=pdt[0:mblk, :])
                        g = ci * 4 + blk
                        nc.vector.max(out=top8_all[0:mblk, g, :],
                                      in_=m8[0:mblk, :])
                # ---- batched tail over all 25 row-blocks at once
                v3a = tp.tile([128, 25, 3], F32, name=f"v3a_{rep}", tag="v3a", bufs=1)
                nc.scalar.activation(v3a[:], top8_all[:, :, 0:3], AF.Sqrt,
                                     scale=-1.0)
                e3a = tp.tile([128, 25, 3], F32, name=f"e3a_{rep}", tag="e3a", bufs=1)
                nc.scalar.activation(e3a[:], v3a[:], AF.Exp, scale=-1.0)
                den = tp.tile([128, 25], F32, name=f"den_{rep}", tag="den", bufs=1)
                nc.vector.reduce_sum(den[:], e3a[:], axis=AX.X)
                rec = tp.tile([128, 25], F32, name=f"rec_{rep}", tag="rec", bufs=1)
                nc.vector.reciprocal(rec[:], den[:])
                num = tp.tile([128, 25], F32, name=f"num_{rep}", tag="num", bufs=1)
                nc.vector.tensor_mul(num[:], e3a[:, :, 0], v3a[:, :, 0])
                nc.vector.tensor_mul(score_sb[:, 0:25], num[:], rec[:])
                nc.sync.dma_start(
                    score.ap()[0:3072].rearrange("(n p) -> p n", p=128),
                    score_sb[:, 0:24])
                nc.sync.dma_start(
                    score.ap()[3072:3136].rearrange("(n p) -> p n", p=64),
                    score_sb[0:64, 24:25])


# ------------------------------------------------------------- host side ----
def prep_shared(W, b, C):
    W = np.asarray(W, np.float32)
    b = np.asarray(b, np.float32)
    C = np.asarray(C, np.float32)
    wt_full = (W[:, :CO].T / 9.0).astype(ml_dtypes.bfloat16)      # (1792, 1792)
    wt = np.ascontiguousarray(
        wt_full.reshape(MCH, 128, CO))
    wt2 = np.ascontiguousarray(W[:, CO:CO + 2].T).astype(ml_dtypes.bfloat16)
    bias = np.ascontiguousarray(b.reshape(MCH, 128))
    c2 = np.ascontiguousarray(
        (2.0 * C).astype(ml_dtypes.bfloat16).reshape(MCH, 128, NCEN))
    cn = (C.astype(np.float64) ** 2).sum(axis=0)
    hi = cn.astype(ml_dtypes.bfloat16)
    lo = (cn - hi.astype(np.float64)).astype(ml_dtypes.bfloat16)
    caug = np.stack([
        -np.ones(NCEN, ml_dtypes.bfloat16),
        -np.ones(NCEN, ml_dtypes.bfloat16),
        -hi, -lo]).astype(ml_dtypes.bfloat16)
    lin = np.linspace(-1.0, 1.0, 56, dtype=np.float32)
    xx = np.broadcast_to(lin[None, :], (56, 56)).reshape(HW)
    yy = np.broadcast_to(lin[:, None], (56, 56)).reshape(HW)
    coords = np.stack([xx, yy]).astype(ml_dtypes.bfloat16)
    return {"wt": wt, "wt2": wt2, "bias": bias, "c2": c2, "caug": caug,
            "coords": coords}


def make_in_maps(p1, p2, p3, W, b, C):
    shared = prep_shared(W, b, C)
    maps = []
    for i in range(N_CORES):
        m = dict(shared)
        m["p1"] = np.ascontiguousarray(np.asarray(p1)[i], np.float32).reshape(256, HW)
        m["p2"] = np.ascontiguousarray(np.asarray(p2)[i], np.float32).reshape(512, 784)
        m["p3"] = np.ascontiguousarray(np.asarray(p3)[i], np.float32).reshape(1024, 196)
        maps.append(m)
    return maps


# ------------------------------------------------------------- runner -------
class _Runtime:
    def __init__(self, repeat=1):
        import jax
        from jax.sharding import Mesh, PartitionSpec
        from jax.experimental.shard_map import shard_map
        from concourse import bass2jax

        self.jax = jax
        self.nc = build_program(repeat=repeat)
        nc = self.nc
        bass2jax.install_neuronx_cc_hook()

        partition_name = (nc.partition_id_tensor.name
                          if nc.partition_id_tensor else None)
        in_names, out_names, out_avals, zero_outs = [], [], [], []
        for alloc in nc.m.functions[0].allocations:
            if not isinstance(alloc, mybir.MemoryLocationSet):
                continue
            name = alloc.memorylocations[0].name
            if alloc.kind == "ExternalInput":
                if name != partition_name:
                    in_names.append(name)
            elif alloc.kind == "ExternalOutput":
                shape = tuple(alloc.tensor_shape)
                dtype = mybir.dt.np(alloc.dtype)
                out_names.append(name)
                out_avals.append(jax.core.ShapedArray(shape, dtype))
                zero_outs.append(np.zeros(shape, dtype))
        self.in_names = list(in_names)
        self.out_names = out_names
        self.out_avals = out_avals
        self.zero_outs = zero_outs
        n_params = len(in_names)
        n_outs = len(out_avals)
        all_in_names = in_names + out_names
        if partition_name is not None:
            all_in_names.append(partition_name)

        def _body(*args):
            operands = list(args)
            if partition_name is not None:
                operands.append(bass2jax.partition_id_tensor())
            outs = bass2jax._bass_exec_p.bind(
                *operands,
                out_avals=tuple(out_avals),
                in_names=tuple(all_in_names),
                out_names=tuple(out_names),
                lowering_input_output_aliases=(),
                sim_require_finite=True,
                sim_require_nnan=True,
                nc=nc,
            )
            return tuple(outs)

        devices = jax.devices()[:N_CORES]
        mesh = Mesh(np.asarray(devices), ("core",))
        self.mesh = mesh
        self.pspec = PartitionSpec("core")
        in_specs = (PartitionSpec("core"),) * (n_params + n_outs)
        out_specs = (PartitionSpec("core"),) * n_outs
        self.sharded = jax.jit(
            shard_map(_body, mesh=mesh, in_specs=in_specs,
                      out_specs=out_specs, check_rep=False),
            donate_argnums=tuple(range(n_params, n_params + n_outs)),
            keep_unused=True,
        )

    def concat_inputs(self, in_maps):
        return [np.concatenate([np.asarray(in_maps[c][nm])
                                for c in range(N_CORES)], axis=0)
                for nm in self.in_names]

    def zeros(self):
        return [np.zeros((N_CORES * z.shape[0], *z.shape[1:]), z.dtype)
                for z in self.zero_outs]

    def device_put_sharded(self, arrays):
        from jax.sharding import NamedSharding
        sh = NamedSharding(self.mesh, self.pspec)
        return [self.jax.device_put(a, sh) for a in arrays]

    def make_chained(self, n_iter):
        """A jitted callable running the kernel n_iter times back-to-back in
        ONE dispatch, chaining the donated output buffers so the executions
        serialize. Used to amortize host/axon dispatch overhead when timing."""
        import jax
        from jax.experimental.shard_map import shard_map
        from jax.sharding import PartitionSpec
        from concourse import bass2jax
        nc = self.nc
        partition_name = (nc.partition_id_tensor.name
                          if nc.partition_id_tensor else None)
        out_avals = self.out_avals
        out_names = self.out_names
        in_names = self.in_names + out_names
        if partition_name is not None:
            in_names_all = in_names + [partition_name]
        else:
            in_names_all = in_names
        n_params = len(self.in_names)
        n_outs = len(out_names)

        def _body(*args):
            params = list(args[:n_params])
            zs = list(args[n_params:])
            for _ in range(n_iter):
                operands = params + zs
                if partition_name is not None:
                    operands.append(bass2jax.partition_id_tensor())
                zs = list(bass2jax._bass_exec_p.bind(
                    *operands,
                    out_avals=tuple(out_avals),
                    in_names=tuple(in_names_all),
                    out_names=tuple(out_names),
                    lowering_input_output_aliases=(),
                    sim_require_finite=True,
                    sim_require_nnan=True,
                    nc=nc,
                ))
            return tuple(zs)

        return jax.jit(
            shard_map(_body, mesh=self.mesh,
                      in_specs=(PartitionSpec("core"),) * (n_params + n_outs),
                      out_specs=(PartitionSpec("core"),) * n_outs,
                      check_rep=False),
            donate_argnums=tuple(range(n_params, n_params + n_outs)),
            keep_unused=True,
        )

    def run(self, in_maps):
        outs = self.sharded(*self.concat_inputs(in_maps), *self.zeros())
        res = {}
        for i, nm in enumerate(self.out_names):
            a = np.asarray(outs[i])
            res[nm] = a.reshape(N_CORES, *self.out_avals[i].shape)
        return res


_RT = {}


def _runtime(repeat=1):
    if repeat not in _RT:
        _RT[repeat] = _Runtime(repeat=repeat)
    return _RT[repeat]


def kernel(p1, p2, p3, W, b, C):
    rt = _runtime()
    in_maps = make_in_maps(p1, p2, p3, W, b, C)
    res = rt.run(in_maps)
    return np.ascontiguousarray(
        res["score"].reshape(N_CORES, 1, 56, 56).astype(np.float32))
